# revision 27
# baseline (speedup 1.0000x reference)
"""Distributed Trainium2 kernel for nn_AttentionLayer (B=2, S=2048, D=2048, H=16).

Sharding: core c = (batch b, head-group g) with b = c // 4, g = c % 4.
Each core owns 4 heads (512 of the 2048 projection dims) of one batch element:
projections (bf16 matmuls, f32 accumulation), masked softmax attention for its
4 heads (no max-subtraction; masked entries become 0 via exp(s)*(1-mask)),
and its partial output projection (Wo row-shard). The 4 partial outputs per
batch are summed on the host (cross-core collectives hang on the axon PJRT
path in this container).

v3 changes (vs v2, driven by trace analysis: DMA front saturated at ~343GB/s
through t=50us with PE starving on activation arrival; 4us A->B boundary gap
from mask DMAs gated on stream-pool teardown; tail epilogue fixed):
- kp/vp are emitted as raw bf16 SBUF dumps (kp_raw straight from kp_sb,
  vpo_raw straight from vpo_sb); the host unpacks/casts. This removes all
  f32 staging copies and halves Phase-A outbound traffic.
- Phase A streams in exact consumption order: wq is interleaved with group-0
  activation tiles on sync/scalar; wk/wv/wo ride sync/scalar interleaved
  behind the NEXT group's activations (not gpsimd up-front), keeping the
  DMA-bound front window free of not-yet-needed bytes. gpsimd carries only
  outputs.
- Mask tiles are [128,2048] (4 per query block) in a dedicated pool created
  BEFORE the Phase-A pools, so mask DMAs are not gated on Phase-A teardown;
  blocks t=0,1 preload during Phase A's back half, t=2,3 load during Phase B.
- Phase B mask multiply is one 2048-wide DVE op per j-pair (2x 16-bit mode).
- Last v-group evictions alternate engines per-copy so the A->B boundary
  drains ~2x faster.
"""

import numpy as np
import ml_dtypes

import concourse.bass as bass  # noqa: F401
import concourse.mybir as mybir
import concourse.tile as tile
from concourse import bacc
from concourse import bass_utils
from concourse.masks import make_identity

BF16 = mybir.dt.bfloat16
F32 = mybir.dt.float32
nbf16 = ml_dtypes.bfloat16

B, S, D, H = 2, 2048, 2048, 16
GH = 4                # heads per core
DH = 128              # head dim
GD = GH * DH          # 512 local projection dims
KC = D // 128         # 16 contraction chunks
NB = 4                # query blocks
BLK = S // NB         # 512
NJ = KC // 2          # 8 skc-pairs per step
N_CORES = 8
SCALE = float(1.0 / np.sqrt(DH))

_CACHE = {}


def _build():
    nc = bacc.Bacc(
        "TRN2", target_bir_lowering=False, debug=False, num_devices=N_CORES
    )
    AF = mybir.ActivationFunctionType

    qT = nc.dram_tensor("qT", [D, S], BF16, kind="ExternalInput")
    kT = nc.dram_tensor("kT", [D, S], BF16, kind="ExternalInput")
    vT = nc.dram_tensor("vT", [D, S], BF16, kind="ExternalInput")
    # mask tile (t, j2) = [128, 2048]: col c2*512+g <-> key (4*j2+c2)*128+p,
    # query t*512+g (value 1.0 where attention allowed, 0.0 where masked)
    maskP = nc.dram_tensor("maskP", [S // 4, 4 * S], BF16, kind="ExternalInput")
    wqP = nc.dram_tensor("wqP", [128, KC * GD], BF16, kind="ExternalInput")
    wkP = nc.dram_tensor("wkP", [128, KC * GD], BF16, kind="ExternalInput")
    wvP = nc.dram_tensor("wvP", [128, KC * GD], BF16, kind="ExternalInput")
    woP = nc.dram_tensor("woP", [128, GH * D], BF16, kind="ExternalInput")
    kp_raw = nc.dram_tensor("kp_raw", [128, GH * S], BF16, kind="ExternalOutput")
    vpo_raw = nc.dram_tensor("vpo_raw", [128, KC * 516], BF16,
                             kind="ExternalOutput")
    outp_out = nc.dram_tensor("outp_out", [S, D], BF16, kind="ExternalOutput")

    with tile.TileContext(nc) as tc:
        with (
            tc.tile_pool(name="res", bufs=1) as res,
            tc.tile_pool(name="mpool", bufs=7) as mpool,
        ):
            # ---- resident SBUF tensors (live across both phases) ----
            wo_sb = res.tile([128, GH * D], BF16, name="wo_sb", tag="wo")
            qp_sb = res.tile([128, GH * S], BF16, name="qp_sb", tag="qp")
            kp_sb = res.tile([128, GH * S], BF16, name="kp_sb", tag="kp")
            # vp + per-head ones column: s-chunk sc at cols sc*516, head h at
            # +h*129 (128 vp dims then one 1.0 column for the softmax denom)
            vpo_sb = res.tile([128, KC * 516], BF16, name="vpo_sb", tag="vpo")
            ident = res.tile([128, 128], BF16, name="ident", tag="ident")

            mask_tiles = {}    # t -> list of 4 [128,2048] tiles

            def emit_mask_dmas(t, alternate=False, j2s=range(4)):
                mts = mask_tiles.setdefault(t, [])
                for j2 in j2s:
                    mt = mpool.tile([128, 2048], BF16, name="mt", tag="mt")
                    eng = nc.scalar if (alternate and j2 % 2 == 1) else nc.sync
                    eng.dma_start(
                        out=mt[:],
                        in_=maskP[j2 * 128:(j2 + 1) * 128,
                                  t * 2048:(t + 1) * 2048],
                    )
                    mts.append(mt)

            # ---------------- Phase A: projections ----------------
            with (
                tc.tile_pool(name="wpool", bufs=1) as wpool,
                tc.tile_pool(name="stream", bufs=33) as stream,
                tc.tile_pool(name="psA", bufs=8, space="PSUM") as psA,
            ):
                wq_sb = wpool.tile([128, KC * GD], BF16, name="wq_sb", tag="wq")
                wk_sb = wpool.tile([128, KC * GD], BF16, name="wk_sb", tag="wk")
                wv_sb = wpool.tile([128, KC * GD], BF16, name="wv_sb", tag="wv")

                # Warm the PE clock gate (HAM) with dummy matmuls while the
                # first activation/weight DMAs are in flight: the PE would
                # otherwise idle ~10us and run its first ~3.4us of real
                # matmuls at half clock.
                wps = psA.tile([128, 512], F32, name="wps", tag="psa")
                for _ in range(56):
                    nc.tensor.matmul(
                        wps[:, 0:128], lhsT=ident[:], rhs=ident[:],
                        start=True, stop=True,
                    )

                make_identity(nc, ident[:])
                nc.gpsimd.memset(vpo_sb[:], 1.0)

                # Projection groups, order q -> k -> v: q produces no output
                # traffic, so the DMA-saturated front window carries only
                # inputs; v's output burst (vpo_raw) lands at the end where
                # the inbound stream is light.
                groups = []
                for xk in ("q", "k", "v"):
                    for npair in range(2):
                        groups.append((xk, npair))
                xdram_of = {"v": vT, "k": kT, "q": qT}
                xss_of = {}

                # Weight/mask staging: emitted interleaved behind the act
                # tiles of the group listed here (queue-FIFO order == arrival
                # order; each consumer waits only for DMAs up to its own).
                # wk lands during g1 compute (needed at g2), wv during g2
                # (needed g4), wo during g3 (needed in phase B), masks t0/t1
                # during g4/g5 (needed at phase B steps 0/4).
                def wchunks(w_sb_, wP_, n):
                    step = (KC * GD) // n
                    return [(w_sb_[:, i * step:(i + 1) * step],
                             wP_[:, i * step:(i + 1) * step]) for i in range(n)]

                extra_of = {
                    2: wchunks(wk_sb, wkP, 4),
                    3: wchunks(wv_sb, wvP, 4),
                    4: wchunks(wo_sb, woP, 4),
                }

                def issue_acts(gi):
                    xk, npair = groups[gi]
                    xd = xdram_of[xk]
                    xss = xss_of.setdefault(gi, [])
                    extras = extra_of.get(gi, [])
                    for kc in range(KC):
                        eng = nc.sync if kc % 2 == 0 else nc.scalar
                        xs = stream.tile([128, 1024], BF16, name="xs", tag="xs")
                        eng.dma_start(
                            out=xs[:],
                            in_=xd[kc * 128:(kc + 1) * 128,
                                   npair * 1024:(npair + 1) * 1024],
                        )
                        xss.append(xs)
                        # interleave a pending weight chunk every 4th tile
                        if kc % 4 == 3 and extras:
                            dst, src = extras.pop(0)
                            weng = nc.sync if (kc // 4) % 2 == 0 else nc.scalar
                            weng.dma_start(out=dst, in_=src)

                def emit_group(gi, xk, npair, xss):
                    # kc-outer over the whole group: 8 [128,512] psum chains
                    # (slot = m*2+h2 for q/k, sl = s-subchunk for v) accumulate
                    # together, consuming one act tile every ~1.7us.  Uniform
                    # ~220GB/s demand -- no per-chain 4.2MB burst, tiles
                    # release incrementally, the stream never falls behind.
                    pss = [psA.tile([128, 512], F32, name="ps", tag="psa")
                           for _ in range(8)]
                    for kc in range(KC):
                        for sl in range(8):
                            if xk == "v":
                                nc.tensor.matmul(
                                    pss[sl][:],
                                    lhsT=xss[kc][:, sl * 128:(sl + 1) * 128],
                                    rhs=wv_sb[:, kc * GD:(kc + 1) * GD],
                                    start=(kc == 0),
                                    stop=(kc == KC - 1),
                                )
                            else:
                                m, h2 = sl // 2, sl % 2
                                wsb = wk_sb if xk == "k" else wq_sb
                                nc.tensor.matmul(
                                    pss[sl][:],
                                    lhsT=wsb[:, kc * GD + m * 128:
                                             kc * GD + (m + 1) * 128],
                                    rhs=xss[kc][:, h2 * 512:(h2 + 1) * 512],
                                    start=(kc == 0),
                                    stop=(kc == KC - 1),
                                )
                    # evictions in allocation order so the next group's psum
                    # reuse unblocks tile-by-tile
                    dst_bf = kp_sb if xk == "k" else qp_sb
                    for sl in range(8):
                        m, h2 = sl // 2, sl % 2
                        dst = dst_bf[:, m * S + npair * 1024 + h2 * 512:
                                     m * S + npair * 1024 + (h2 + 1) * 512]
                        if m % 2 == 0:
                            nc.scalar.copy(dst, pss[sl][:])
                        else:
                            nc.vector.tensor_copy(dst, pss[sl][:])

                def emit_v_chain(mqp, sp, xss):
                    # v groups run m-outer (their data is fully resident by
                    # then -- the stream is ~60us ahead) so the 32 vpo
                    # evictions stagger chain-by-chain instead of bunching
                    # after the group's last matmul, which would stall the
                    # phase boundary and phase B's first ctx matmuls.
                    ps2 = [psA.tile([128, 512], F32, name="ps", tag="psa")
                           for _ in range(2)]
                    for kc in range(KC):
                        for h2 in range(2):
                            sl = sp * 2 + h2
                            nc.tensor.matmul(
                                ps2[h2][:],
                                lhsT=xss[kc][:, sl * 128:(sl + 1) * 128],
                                rhs=wv_sb[:, kc * GD:(kc + 1) * GD],
                                start=(kc == 0),
                                stop=(kc == KC - 1),
                            )
                    # whole-chain engine alternation: cross-engine writes to
                    # vpo_sb serialize (tile-granular WAW ordering)
                    for h2 in range(2):
                        sc = mqp * 8 + sp * 2 + h2
                        for h in range(GH):
                            dst = vpo_sb[:, sc * 516 + h * 129:
                                         sc * 516 + h * 129 + 128]
                            src = ps2[h2][:, h * 128:(h + 1) * 128]
                            if sp % 2 == 0:
                                nc.scalar.copy(dst, src)
                            else:
                                nc.vector.tensor_copy(dst, src)

                # group 0: wq chunks interleaved with its own act tiles in
                # exact consumption order (kc-outer chains below tolerate the
                # cold stream, consuming one tile every ~1.7us); kc-granular
                # wq chunks so the first matmul starts one chunk earlier
                xss0 = xss_of.setdefault(0, [])
                for kc in range(KC):
                    eng = nc.sync if kc % 2 == 0 else nc.scalar
                    if kc % 2 == 0:
                        # [128,1024] wq pair-chunk (2KB per partition row
                        # keeps DMA burst efficiency), queues alternating
                        weng = nc.sync if kc % 4 == 0 else nc.scalar
                        weng.dma_start(
                            out=wq_sb[:, kc * GD:(kc + 2) * GD],
                            in_=wqP[:, kc * GD:(kc + 2) * GD],
                        )
                    xs = stream.tile([128, 1024], BF16, name="xs", tag="xs")
                    eng.dma_start(
                        out=xs[:],
                        in_=qT[kc * 128:(kc + 1) * 128, 0:1024],
                    )
                    xss0.append(xs)

                for gi, (xk, npair) in enumerate(groups):
                    xss = xss_of[gi]
                    if gi + 1 < len(groups):
                        issue_acts(gi + 1)
                    if gi == 4:
                        emit_mask_dmas(0, alternate=True)
                    if gi == 5:
                        # only 3 of block 1's mask tiles fit in mpool
                        # alongside block 0's; the 4th loads in phase B
                        emit_mask_dmas(1, alternate=True, j2s=range(3))
                    if xk == "v":
                        for sp in range(4):
                            emit_v_chain(npair, sp, xss)
                    else:
                        emit_group(gi, xk, npair, xss)
                    if xk == "k" and npair == 1:
                        # kp complete: dump raw bf16 (host unpacks);
                        # gpsimd queue is otherwise idle until phase B
                        for half in range(2):
                            nc.gpsimd.dma_start(
                                out=kp_raw[:, half * GH * S // 2:
                                           (half + 1) * GH * S // 2],
                                in_=kp_sb[:, half * GH * S // 2:
                                          (half + 1) * GH * S // 2],
                            )
                # (vpo_raw is dumped from phase B's pipeline: emitting it
                # here would park a long gating wait on the gpsimd queue)

            # ---------------- Phase B: attention + out-proj ----------------
            with (
                tc.tile_pool(name="apl", bufs=9) as apl,
                tc.tile_pool(name="cpl", bufs=10) as cpl,
                tc.tile_pool(name="stageB", bufs=12) as stageB,
                tc.tile_pool(name="ostp", bufs=6) as ostp,
                tc.tile_pool(name="psS", bufs=2, space="PSUM") as psS,
                tc.tile_pool(name="psCT", bufs=2, space="PSUM") as psCT,
                tc.tile_pool(name="psOT", bufs=2, space="PSUM") as psOT,
            ):
                steps = [(t, h) for t in range(NB) for h in range(GH)]
                at2_tiles = {}     # (i, j2) -> [128,2048] tile (skc 4j2..+3)
                cps_tiles = {}     # (i, jj//2) -> cps tile holding 2 chunks
                ctn_tiles = {}     # (i, jj) -> normalized ctx [128,128]
                cth_tiles = {}     # (t, h) -> [128,512] transposed ctx

                def emit_scores_pair(i, j):
                    t, h = steps[i]
                    sps = psS.tile([128, 1024], F32, name="sps", tag="sps")
                    for c in range(2):
                        skc = 2 * j + c
                        nc.tensor.matmul(
                            sps[:, c * 512:(c + 1) * 512],
                            lhsT=kp_sb[:, h * S + skc * 128:
                                       h * S + (skc + 1) * 128],
                            rhs=qp_sb[:, h * S + t * 512: h * S + (t + 1) * 512],
                            start=True,
                            stop=True,
                        )
                    j2 = j // 2
                    if j % 2 == 0:
                        at2_tiles[(i, j2)] = apl.tile(
                            [128, 2048], BF16, name="at2", tag="at2")
                    at2 = at2_tiles[(i, j2)]
                    half = at2[:, (j % 2) * 1024:(j % 2) * 1024 + 1024]
                    nc.scalar.activation(half, sps[:], AF.Exp, scale=SCALE)
                    if j % 2 == 1:
                        # one 2048-wide mask multiply per j-pair (DVE 2x mode)
                        nc.vector.tensor_mul(
                            at2[:], at2[:], mask_tiles[t][j2][:])

                def emit_ctx_part(i, j):
                    # ctx matmuls for step i, slice j: mm-chunk jj = j//2,
                    # skc range (j%2)*8 .. +8; finalize (recip+norm) at odd j.
                    t, h = steps[i]
                    jj = j // 2
                    if j % 2 == 0 and jj % 2 == 0:
                        cps_tiles[(i, jj // 2)] = psCT.tile(
                            [128, 512], F32, name="cps", tag="cps")
                    cps = cps_tiles[(i, jj // 2)]
                    off = (jj % 2) * 129
                    for skc in range((j % 2) * 8, (j % 2) * 8 + 8):
                        at2 = at2_tiles[(i, skc // 4)]
                        nc.tensor.matmul(
                            cps[:, off:off + 129],
                            lhsT=at2[:, (skc % 4) * 512 + jj * 128:
                                     (skc % 4) * 512 + (jj + 1) * 128],
                            rhs=vpo_sb[:, skc * 516 + h * 129:
                                       skc * 516 + (h + 1) * 129],
                            start=(skc == 0),
                            stop=(skc == KC - 1),
                        )
                    if j % 2 == 1:
                        rec = stageB.tile([128, 1], F32, name="rec", tag="rec")
                        nc.vector.reciprocal(rec[:], cps[:, off + 128:off + 129])
                        ctn = stageB.tile([128, 128], BF16, name="ctn",
                                          tag="ctn")
                        nc.vector.tensor_scalar_mul(
                            ctn[:], cps[:, off:off + 128], rec[:])
                        ctn_tiles[(i, jj)] = ctn
                    if j == 7:
                        # release the at2 tiles of step i
                        for jd in range(4):
                            at2_tiles.pop((i, jd), None)

                def emit_transposes(i):
                    t, h = steps[i]
                    tps = psOT.tile([128, 512], BF16, name="tps", tag="psot")
                    for mm in range(4):
                        nc.tensor.transpose(
                            tps[:, mm * 128:(mm + 1) * 128],
                            ctn_tiles.pop((i, mm))[:], ident[:])
                    cth = cpl.tile([128, 512], BF16, name="cth", tag="cth")
                    nc.vector.tensor_copy(cth[:], tps[:])
                    cth_tiles[(t, h)] = cth

                def emit_outproj_unit(t, mm, npair, final=False):
                    # one (mm, npair) unit: 8 matmuls accumulating over the
                    # 4 heads into a [128,1024] output row-block slice
                    if final:
                        # scores are done: run through the freed psS pool
                        # ([128,1024] pairs, 2-deep) so units pipeline
                        op = psS.tile([128, 1024], F32, name="opw", tag="sps")
                        ops2 = [op[:, 0:512], op[:, 512:1024]]
                    else:
                        ops2 = [psOT.tile([128, 512], F32,
                                          name=f"ops{j2}", tag="psot")
                                for j2 in range(2)]
                    for h in range(GH):
                        for n2 in range(2):
                            n = npair * 2 + n2
                            nc.tensor.matmul(
                                ops2[n2][:],
                                lhsT=cth_tiles[(t, h)][:,
                                                       mm * 128:(mm + 1) * 128],
                                rhs=wo_sb[:, h * D + n * 512:
                                          h * D + (n + 1) * 512],
                                start=(h == 0),
                                stop=(h == GH - 1),
                            )
                    if mm == 3 and npair == 1:
                        for h in range(GH):
                            cth_tiles.pop((t, h))
                    ost = ostp.tile([128, 1024], BF16, name="ost", tag="ost")
                    if final:
                        # contiguous [128,1024] psum pair: one full-tile
                        # eviction, engines alternating
                        if (mm + npair) % 2 == 0:
                            nc.scalar.copy(ost[:], op[:])
                        else:
                            nc.vector.tensor_copy(ost[:], op[:])
                        # split across both queues so the tail drains fast
                        nc.sync.dma_start(
                            out=outp_out[t * BLK + mm * 128:
                                         t * BLK + (mm + 1) * 128,
                                         npair * 1024:npair * 1024 + 512],
                            in_=ost[:, 0:512],
                        )
                        nc.gpsimd.dma_start(
                            out=outp_out[t * BLK + mm * 128:
                                         t * BLK + (mm + 1) * 128,
                                         npair * 1024 + 512:
                                         (npair + 1) * 1024],
                            in_=ost[:, 512:1024],
                        )
                    else:
                        nc.scalar.copy(ost[:, 0:512], ops2[0][:])
                        nc.vector.tensor_copy(ost[:, 512:1024], ops2[1][:])
                        eng = nc.sync if npair == 0 else nc.gpsimd
                        eng.dma_start(
                            out=outp_out[t * BLK + mm * 128:
                                         t * BLK + (mm + 1) * 128,
                                         npair * 1024:(npair + 1) * 1024],
                            in_=ost[:],
                        )

                # -------- software pipeline --------
                # scores pairs 2-at-a-time, ctx in runs of 16 matmuls so the
                # PE weight-buffer pipeline stays dense within each class.
                # Out-proj units (8 matmuls each) are spread 2-per-step so
                # the PE stays the bottleneck on every step: bulk per-block
                # out-proj left ScalarE (8 exps = 8.9us) as the limiter on
                # the 3-of-4 steps that had no out-proj work.
                pending = []       # outproj units ready to emit
                for i in range(len(steps) + 1):
                    t, h = steps[i] if i < len(steps) else (None, None)
                    fresh = False  # block enqueued this step: its last cth
                    for jj in range(4):  # CAST is only one slot old at jj=1
                        if i < len(steps):
                            emit_scores_pair(i, 2 * jj)
                            emit_scores_pair(i, 2 * jj + 1)
                        if jj == 0 and i >= 2:
                            emit_transposes(i - 2)
                            tb, hb = steps[i - 2]
                            if hb == 3:
                                pending += [(tb, mm, npair)
                                            for mm in range(4)
                                            for npair in range(2)]
                                fresh = True
                        if jj == 0 and i == 1:
                            emit_mask_dmas(1, j2s=(3,))
                            # vp complete: dump raw bf16 (gating wait is
                            # already satisfied here, so the gpsimd queue
                            # is not blocked)
                            for half in range(2):
                                nc.gpsimd.dma_start(
                                    out=vpo_raw[:, half * KC * 516 // 2:
                                                (half + 1) * KC * 516 // 2],
                                    in_=vpo_sb[:, half * KC * 516 // 2:
                                               (half + 1) * KC * 516 // 2],
                                )
                        if jj == 1 and i < len(steps) and h == 2 and \
                                t + 1 < NB and t + 1 >= 2:
                            emit_mask_dmas(t + 1)
                        if jj == 3 or (jj == 1 and not fresh):
                            if pending:
                                emit_outproj_unit(*pending.pop(0))
                        if i >= 1:
                            emit_ctx_part(i - 1, 2 * jj)
                            emit_ctx_part(i - 1, 2 * jj + 1)
                # epilogue: transposes of the last step, remaining outproj
                # (block 3 plus any leftovers) through the freed psS pool
                emit_transposes(len(steps) - 1)
                pending += [(NB - 1, mm, npair)
                            for mm in range(4) for npair in range(2)]
                for unit in pending:
                    emit_outproj_unit(*unit, final=True)

    nc.compile()
    return nc


def get_nc():
    if "nc" not in _CACHE:
        _CACHE["nc"] = _build()
    return _CACHE["nc"]


def make_in_maps(inputs):
    q = np.asarray(inputs["q"], np.float32)
    k = np.asarray(inputs["k"], np.float32)
    v = np.asarray(inputs["v"], np.float32)
    mask = np.asarray(inputs["mask"])
    Wq = np.asarray(inputs["Wq"], np.float32)
    Wk = np.asarray(inputs["Wk"], np.float32)
    Wv = np.asarray(inputs["Wv"], np.float32)
    Wo = np.asarray(inputs["Wo"], np.float32)

    per_batch = []
    for b in range(B):
        maskTb = np.ascontiguousarray(
            (~mask[b].astype(bool)).T).astype(nbf16)  # [key, q]
        # tile (t, j2) = [128, 2048]: [j2*128+p, t*2048 + c2*512 + g]
        #   = maskTb[(4*j2+c2)*128 + p, t*512 + g]
        maskP = np.ascontiguousarray(
            maskTb.reshape(4, 4, 128, NB, 512)
            .transpose(0, 2, 3, 1, 4).reshape(S // 4, 4 * S))
        per_batch.append({
            "qT": np.ascontiguousarray(q[b].T).astype(nbf16),
            "kT": np.ascontiguousarray(k[b].T).astype(nbf16),
            "vT": np.ascontiguousarray(v[b].T).astype(nbf16),
            "maskP": maskP,
        })

    def packw(wT, ncols):
        # wT [rows, ncols] -> [128, (rows//128)*ncols] with chunk-major cols
        r = wT.shape[0] // 128
        return np.ascontiguousarray(
            wT.reshape(r, 128, ncols).transpose(1, 0, 2).reshape(128, r * ncols))

    per_group = []
    for g in range(4):
        sl = slice(g * GD, (g + 1) * GD)
        per_group.append({
            "wqP": packw(np.ascontiguousarray(Wq[sl, :].T).astype(nbf16), GD),
            "wkP": packw(np.ascontiguousarray(Wk[sl, :].T).astype(nbf16), GD),
            "wvP": packw(np.ascontiguousarray(Wv[sl, :].T).astype(nbf16), GD),
            "woP": packw(np.ascontiguousarray(Wo[:, sl].T).astype(nbf16), D),
        })
    in_maps = []
    for c in range(N_CORES):
        b, g = c // 4, c % 4
        m = {}
        m.update(per_batch[b])
        m.update(per_group[g])
        in_maps.append(m)
    return in_maps


def assemble(results):
    out = np.zeros((B, S, D), np.float32)
    kp = np.empty((B, S, D), np.float32)
    vp = np.empty((B, S, D), np.float32)
    for c, res in enumerate(results):
        b, g = c // 4, c % 4
        # kp_raw[p, m*2048 + s] = kp[s, g*512 + m*128 + p]
        kpr = np.asarray(res["kp_raw"]).astype(np.float32)
        kp[b][:, g * GD:(g + 1) * GD] = (
            kpr.reshape(128, 4, S).transpose(2, 1, 0).reshape(S, GD))
        # vpo_raw[p, sc*516 + h*129 + c] = vp[sc*128 + p, g*512 + h*128 + c]
        vpr = np.asarray(res["vpo_raw"]).astype(np.float32)
        vpr = vpr.reshape(128, KC, GH, 129)[:, :, :, :128]
        vp[b][:, g * GD:(g + 1) * GD] = (
            vpr.transpose(1, 0, 2, 3).reshape(S, GD))
        out[b] += res["outp_out"].astype(np.float32)
    return out, kp, vp


def run_cores(in_maps, trace=False, **kwargs):
    nc = get_nc()
    return bass_utils.run_bass_kernel_spmd(
        nc, in_maps, core_ids=list(range(N_CORES)), trace=trace, **kwargs
    )


def kernel(**inputs):
    in_maps = make_in_maps(inputs)
    res = run_cores(in_maps, trace=False)
    return assemble(res.results)


# revision 32
# speedup vs baseline: 1.0616x; 1.0616x over previous
"""Distributed Trainium2 kernel for nn_AttentionLayer (B=2, S=2048, D=2048, H=16).

Sharding: core c = (batch b, head-group g) with b = c // 4, g = c % 4.
Each core owns 4 heads (512 of the 2048 projection dims) of one batch element:
projections (bf16 matmuls, f32 accumulation), masked softmax attention for its
4 heads (no max-subtraction; masked entries become 0 via exp(s)*(1-mask)),
and its partial output projection (Wo row-shard). The 4 partial outputs per
batch are summed on the host (cross-core collectives hang on the axon PJRT
path in this container).

v3 changes (vs v2, driven by trace analysis: DMA front saturated at ~343GB/s
through t=50us with PE starving on activation arrival; 4us A->B boundary gap
from mask DMAs gated on stream-pool teardown; tail epilogue fixed):
- kp/vp are emitted as raw bf16 SBUF dumps (kp_raw straight from kp_sb,
  vpo_raw straight from vpo_sb); the host unpacks/casts. This removes all
  f32 staging copies and halves Phase-A outbound traffic.
- Phase A streams in exact consumption order: wq is interleaved with group-0
  activation tiles on sync/scalar; wk/wv/wo ride sync/scalar interleaved
  behind the NEXT group's activations (not gpsimd up-front), keeping the
  DMA-bound front window free of not-yet-needed bytes. gpsimd carries only
  outputs.
- Mask tiles are [128,2048] (4 per query block) in a dedicated pool created
  BEFORE the Phase-A pools, so mask DMAs are not gated on Phase-A teardown;
  blocks t=0,1 preload during Phase A's back half, t=2,3 load during Phase B.
- Phase B mask multiply is one 2048-wide DVE op per j-pair (2x 16-bit mode).
- Last v-group evictions alternate engines per-copy so the A->B boundary
  drains ~2x faster.
"""

import numpy as np
import ml_dtypes

import concourse.bass as bass  # noqa: F401
import concourse.mybir as mybir
import concourse.tile as tile
from concourse import bacc
from concourse import bass_utils
from concourse.masks import make_identity

BF16 = mybir.dt.bfloat16
F32 = mybir.dt.float32
nbf16 = ml_dtypes.bfloat16

B, S, D, H = 2, 2048, 2048, 16
GH = 4                # heads per core
DH = 128              # head dim
GD = GH * DH          # 512 local projection dims
KC = D // 128         # 16 contraction chunks
NB = 4                # query blocks
BLK = S // NB         # 512
NJ = KC // 2          # 8 skc-pairs per step
N_CORES = 8
SCALE = float(1.0 / np.sqrt(DH))

_CACHE = {}


def _build():
    nc = bacc.Bacc(
        "TRN2", target_bir_lowering=False, debug=False, num_devices=N_CORES
    )
    AF = mybir.ActivationFunctionType

    qT = nc.dram_tensor("qT", [D, S], BF16, kind="ExternalInput")
    kT = nc.dram_tensor("kT", [D, S], BF16, kind="ExternalInput")
    vT = nc.dram_tensor("vT", [D, S], BF16, kind="ExternalInput")
    # mask tile (t, j2) = [128, 2048]: col c2*512+g <-> key (4*j2+c2)*128+p,
    # query t*512+g (value 1.0 where attention allowed, 0.0 where masked)
    maskP = nc.dram_tensor("maskP", [S // 4, 4 * S], BF16, kind="ExternalInput")
    wqP = nc.dram_tensor("wqP", [128, KC * GD], BF16, kind="ExternalInput")
    wkP = nc.dram_tensor("wkP", [128, KC * GD], BF16, kind="ExternalInput")
    wvP = nc.dram_tensor("wvP", [128, KC * GD], BF16, kind="ExternalInput")
    woP = nc.dram_tensor("woP", [128, GH * D], BF16, kind="ExternalInput")
    kp_raw = nc.dram_tensor("kp_raw", [128, GH * S], BF16, kind="ExternalOutput")
    vpo_raw = nc.dram_tensor("vpo_raw", [128, KC * 516], BF16,
                             kind="ExternalOutput")
    outp_out = nc.dram_tensor("outp_out", [S, D], BF16, kind="ExternalOutput")

    with tile.TileContext(nc) as tc:
        with (
            tc.tile_pool(name="res", bufs=1) as res,
            tc.tile_pool(name="mpool", bufs=7) as mpool,
        ):
            # ---- resident SBUF tensors (live across both phases) ----
            wo_sb = res.tile([128, GH * D], BF16, name="wo_sb", tag="wo")
            qp_sb = res.tile([128, GH * S], BF16, name="qp_sb", tag="qp")
            kp_sb = res.tile([128, GH * S], BF16, name="kp_sb", tag="kp")
            # vp + per-head ones column: s-chunk sc at cols sc*516, head h at
            # +h*129 (128 vp dims then one 1.0 column for the softmax denom)
            vpo_sb = res.tile([128, KC * 516], BF16, name="vpo_sb", tag="vpo")
            ident = res.tile([128, 128], BF16, name="ident", tag="ident")

            mask_tiles = {}    # t -> list of 4 [128,2048] tiles

            def emit_mask_dmas(t, alternate=False, j2s=range(4)):
                mts = mask_tiles.setdefault(t, [])
                for j2 in j2s:
                    mt = mpool.tile([128, 2048], BF16, name="mt", tag="mt")
                    eng = nc.gpsimd if (alternate and j2 % 2 == 1) else nc.sync
                    eng.dma_start(
                        out=mt[:],
                        in_=maskP[j2 * 128:(j2 + 1) * 128,
                                  t * 2048:(t + 1) * 2048],
                    )
                    mts.append(mt)

            # ---------------- Phase A: projections ----------------
            with (
                tc.tile_pool(name="wpool", bufs=1) as wpool,
                tc.tile_pool(name="stream", bufs=41) as stream,
                tc.tile_pool(name="psA", bufs=8, space="PSUM") as psA,
            ):
                # wv is allocated lazily at group 2 sharing wq's TAG with
                # bufs=1: the ring reuses wq's buffer (wq's last consumer is
                # group 1's final matmul), freeing 16KB/partition for a
                # deeper stream pool
                W = {
                    "q": wpool.tile([128, KC * GD], BF16, name="wq_sb",
                                    tag="wqv"),
                    "k": wpool.tile([128, KC * GD], BF16, name="wk_sb",
                                    tag="wk"),
                }

                # Warm the PE clock gate (HAM) with dummy matmuls while the
                # first activation/weight DMAs are in flight: the PE would
                # otherwise idle ~10us and run its first ~3.4us of real
                # matmuls at half clock.
                wps = psA.tile([128, 512], F32, name="wps", tag="psa")
                for _ in range(56):
                    nc.tensor.matmul(
                        wps[:, 0:128], lhsT=ident[:], rhs=ident[:],
                        start=True, stop=True,
                    )

                make_identity(nc, ident[:])
                nc.vector.memset(vpo_sb[:], 1.0)

                # Projection groups, order q -> k -> v: q produces no output
                # traffic, so the DMA-saturated front window carries only
                # inputs; v's output burst (vpo_raw) lands at the end where
                # the inbound stream is light.
                groups = []
                for xk in ("q", "k", "v"):
                    for npair in range(2):
                        groups.append((xk, npair))
                xdram_of = {"v": vT, "k": kT, "q": qT}
                xss_of = {}

                # Weight/mask staging: emitted interleaved behind the act
                # tiles of the group listed here (queue-FIFO order == arrival
                # order; each consumer waits only for DMAs up to its own).
                # wk lands during g1 compute (needed at g2), wv during g2
                # (needed g4), wo during g3 (needed in phase B), masks t0/t1
                # during g4/g5 (needed at phase B steps 0/4).
                def wchunks(w_sb_, wP_, n):
                    step = (KC * GD) // n
                    return [(w_sb_[:, i * step:(i + 1) * step],
                             wP_[:, i * step:(i + 1) * step]) for i in range(n)]

                def extras_for(gi):
                    if gi == 2:
                        return wchunks(W["k"], wkP, 4)
                    if gi == 3:
                        W["v"] = wpool.tile([128, KC * GD], BF16,
                                            name="wv_sb", tag="wqv")
                        return wchunks(W["v"], wvP, 4)
                    if gi == 4:
                        return wchunks(wo_sb, woP, 4)
                    return []

                def issue_acts(gi):
                    xk, npair = groups[gi]
                    xd = xdram_of[xk]
                    xss = xss_of.setdefault(gi, [])
                    extras = extras_for(gi)
                    for kc in range(KC):
                        eng = nc.sync if kc % 2 == 0 else nc.gpsimd
                        xs = stream.tile([128, 1024], BF16, name="xs", tag="xs")
                        eng.dma_start(
                            out=xs[:],
                            in_=xd[kc * 128:(kc + 1) * 128,
                                   npair * 1024:(npair + 1) * 1024],
                        )
                        xss.append(xs)
                        # interleave a pending weight chunk every 4th tile
                        if kc % 4 == 3 and extras:
                            dst, src = extras.pop(0)
                            weng = nc.sync if (kc // 4) % 2 == 0 else nc.gpsimd
                            weng.dma_start(out=dst, in_=src)

                def emit_group(gi, xk, npair, xss):
                    # kc-outer over the whole group: 8 [128,512] psum chains
                    # (slot = m*2+h2 for q/k, sl = s-subchunk for v) accumulate
                    # together, consuming one act tile every ~1.7us.  Uniform
                    # ~220GB/s demand -- no per-chain 4.2MB burst, tiles
                    # release incrementally, the stream never falls behind.
                    pss = [psA.tile([128, 512], F32, name="ps", tag="psa")
                           for _ in range(8)]
                    for kc in range(KC):
                        for sl in range(8):
                            if xk == "v":
                                nc.tensor.matmul(
                                    pss[sl][:],
                                    lhsT=xss[kc][:, sl * 128:(sl + 1) * 128],
                                    rhs=W["v"][:, kc * GD:(kc + 1) * GD],
                                    start=(kc == 0),
                                    stop=(kc == KC - 1),
                                )
                            else:
                                m, h2 = sl // 2, sl % 2
                                wsb = W["k"] if xk == "k" else W["q"]
                                nc.tensor.matmul(
                                    pss[sl][:],
                                    lhsT=wsb[:, kc * GD + m * 128:
                                             kc * GD + (m + 1) * 128],
                                    rhs=xss[kc][:, h2 * 512:(h2 + 1) * 512],
                                    start=(kc == 0),
                                    stop=(kc == KC - 1),
                                )
                    # evictions in allocation order so the next group's psum
                    # reuse unblocks tile-by-tile
                    dst_bf = kp_sb if xk == "k" else qp_sb
                    for sl in range(8):
                        m, h2 = sl // 2, sl % 2
                        dst = dst_bf[:, m * S + npair * 1024 + h2 * 512:
                                     m * S + npair * 1024 + (h2 + 1) * 512]
                        if m % 2 == 0:
                            nc.scalar.copy(dst, pss[sl][:])
                        else:
                            nc.vector.tensor_copy(dst, pss[sl][:])

                def emit_v_chain(mqp, sp, xss):
                    # v groups run m-outer (their data is fully resident by
                    # then -- the stream is ~60us ahead) so the 32 vpo
                    # evictions stagger chain-by-chain instead of bunching
                    # after the group's last matmul, which would stall the
                    # phase boundary and phase B's first ctx matmuls.
                    ps2 = [psA.tile([128, 512], F32, name="ps", tag="psa")
                           for _ in range(2)]
                    for kc in range(KC):
                        for h2 in range(2):
                            sl = sp * 2 + h2
                            nc.tensor.matmul(
                                ps2[h2][:],
                                lhsT=xss[kc][:, sl * 128:(sl + 1) * 128],
                                rhs=W["v"][:, kc * GD:(kc + 1) * GD],
                                start=(kc == 0),
                                stop=(kc == KC - 1),
                            )
                    # whole-chain engine alternation: cross-engine writes to
                    # vpo_sb serialize (tile-granular WAW ordering)
                    for h2 in range(2):
                        sc = mqp * 8 + sp * 2 + h2
                        for h in range(GH):
                            dst = vpo_sb[:, sc * 516 + h * 129:
                                         sc * 516 + h * 129 + 128]
                            src = ps2[h2][:, h * 128:(h + 1) * 128]
                            if sp % 2 == 0:
                                nc.scalar.copy(dst, src)
                            else:
                                nc.vector.tensor_copy(dst, src)

                # group 0: wq chunks interleaved with its own act tiles in
                # exact consumption order (kc-outer chains below tolerate the
                # cold stream, consuming one tile every ~1.7us); kc-granular
                # wq chunks so the first matmul starts one chunk earlier
                xss0 = xss_of.setdefault(0, [])
                for kc in range(KC):
                    eng = nc.sync if kc % 2 == 0 else nc.gpsimd
                    if kc % 2 == 0:
                        # [128,1024] wq pair-chunk (2KB per partition row
                        # keeps DMA burst efficiency), queues alternating
                        weng = nc.sync if kc % 4 == 0 else nc.gpsimd
                        weng.dma_start(
                            out=W["q"][:, kc * GD:(kc + 2) * GD],
                            in_=wqP[:, kc * GD:(kc + 2) * GD],
                        )
                    xs = stream.tile([128, 1024], BF16, name="xs", tag="xs")
                    eng.dma_start(
                        out=xs[:],
                        in_=qT[kc * 128:(kc + 1) * 128, 0:1024],
                    )
                    xss0.append(xs)

                for gi, (xk, npair) in enumerate(groups):
                    xss = xss_of[gi]
                    if gi + 1 < len(groups):
                        issue_acts(gi + 1)
                    if gi == 4:
                        emit_mask_dmas(0, alternate=True)
                    if gi == 5:
                        # only 3 of block 1's mask tiles fit in mpool
                        # alongside block 0's; the 4th loads in phase B
                        emit_mask_dmas(1, alternate=True, j2s=range(3))
                    if xk == "v":
                        for sp in range(4):
                            emit_v_chain(npair, sp, xss)
                    else:
                        emit_group(gi, xk, npair, xss)
                    if xk == "k" and npair == 1:
                        # kp complete: dump raw bf16 (host unpacks);
                        # gpsimd queue is otherwise idle until phase B
                        for half in range(2):
                            nc.gpsimd.dma_start(
                                out=kp_raw[:, half * GH * S // 2:
                                           (half + 1) * GH * S // 2],
                                in_=kp_sb[:, half * GH * S // 2:
                                          (half + 1) * GH * S // 2],
                            )
                # (vpo_raw is dumped from phase B's pipeline: emitting it
                # here would park a long gating wait on the gpsimd queue)

            # ---------------- Phase B: attention + out-proj ----------------
            with (
                tc.tile_pool(name="apl", bufs=9) as apl,
                tc.tile_pool(name="cpl", bufs=10) as cpl,
                tc.tile_pool(name="stageB", bufs=12) as stageB,
                tc.tile_pool(name="ostp", bufs=6) as ostp,
                tc.tile_pool(name="psS", bufs=2, space="PSUM") as psS,
                tc.tile_pool(name="psCT", bufs=2, space="PSUM") as psCT,
                tc.tile_pool(name="psOT", bufs=2, space="PSUM") as psOT,
            ):
                steps = [(t, h) for t in range(NB) for h in range(GH)]
                at2_tiles = {}     # (i, j2) -> [128,2048] tile (skc 4j2..+3)
                cps_tiles = {}     # (i, jj//2) -> cps tile holding 2 chunks
                ctn_tiles = {}     # (i, jj) -> normalized ctx [128,128]
                cth_tiles = {}     # (t, h) -> [128,512] transposed ctx

                def emit_scores_pair(i, j):
                    t, h = steps[i]
                    sps = psS.tile([128, 1024], F32, name="sps", tag="sps")
                    for c in range(2):
                        skc = 2 * j + c
                        nc.tensor.matmul(
                            sps[:, c * 512:(c + 1) * 512],
                            lhsT=kp_sb[:, h * S + skc * 128:
                                       h * S + (skc + 1) * 128],
                            rhs=qp_sb[:, h * S + t * 512: h * S + (t + 1) * 512],
                            start=True,
                            stop=True,
                        )
                    j2 = j // 2
                    if j % 2 == 0:
                        at2_tiles[(i, j2)] = apl.tile(
                            [128, 2048], BF16, name="at2", tag="at2")
                    at2 = at2_tiles[(i, j2)]
                    half = at2[:, (j % 2) * 1024:(j % 2) * 1024 + 1024]
                    nc.scalar.activation(half, sps[:], AF.Exp, scale=SCALE)
                    if j % 2 == 1:
                        # one 2048-wide mask multiply per j-pair (DVE 2x mode)
                        nc.vector.tensor_mul(
                            at2[:], at2[:], mask_tiles[t][j2][:])

                def emit_ctx_part(i, j):
                    # ctx matmuls for step i, slice j: mm-chunk jj = j//2,
                    # skc range (j%2)*8 .. +8; finalize (recip+norm) at odd j.
                    t, h = steps[i]
                    jj = j // 2
                    if j % 2 == 0 and jj % 2 == 0:
                        cps_tiles[(i, jj // 2)] = psCT.tile(
                            [128, 512], F32, name="cps", tag="cps")
                    cps = cps_tiles[(i, jj // 2)]
                    off = (jj % 2) * 129
                    for skc in range((j % 2) * 8, (j % 2) * 8 + 8):
                        at2 = at2_tiles[(i, skc // 4)]
                        nc.tensor.matmul(
                            cps[:, off:off + 129],
                            lhsT=at2[:, (skc % 4) * 512 + jj * 128:
                                     (skc % 4) * 512 + (jj + 1) * 128],
                            rhs=vpo_sb[:, skc * 516 + h * 129:
                                       skc * 516 + (h + 1) * 129],
                            start=(skc == 0),
                            stop=(skc == KC - 1),
                        )
                    if j % 2 == 1:
                        rec = stageB.tile([128, 1], F32, name="rec", tag="rec")
                        nc.vector.reciprocal(rec[:], cps[:, off + 128:off + 129])
                        ctn = stageB.tile([128, 128], BF16, name="ctn",
                                          tag="ctn")
                        nc.vector.tensor_scalar_mul(
                            ctn[:], cps[:, off:off + 128], rec[:])
                        ctn_tiles[(i, jj)] = ctn
                    if j == 7:
                        # release the at2 tiles of step i
                        for jd in range(4):
                            at2_tiles.pop((i, jd), None)

                def emit_transposes(i):
                    t, h = steps[i]
                    tps = psOT.tile([128, 512], BF16, name="tps", tag="psot")
                    for mm in range(4):
                        nc.tensor.transpose(
                            tps[:, mm * 128:(mm + 1) * 128],
                            ctn_tiles.pop((i, mm))[:], ident[:])
                    cth = cpl.tile([128, 512], BF16, name="cth", tag="cth")
                    nc.vector.tensor_copy(cth[:], tps[:])
                    cth_tiles[(t, h)] = cth

                def emit_outproj_unit(t, mm, npair, final=False):
                    # one (mm, npair) unit: 8 matmuls accumulating over the
                    # 4 heads into a [128,1024] output row-block slice
                    if final:
                        # scores are done: run through the freed psS pool
                        # ([128,1024] pairs, 2-deep) so units pipeline
                        op = psS.tile([128, 1024], F32, name="opw", tag="sps")
                        ops2 = [op[:, 0:512], op[:, 512:1024]]
                    else:
                        ops2 = [psOT.tile([128, 512], F32,
                                          name=f"ops{j2}", tag="psot")
                                for j2 in range(2)]
                    for h in range(GH):
                        for n2 in range(2):
                            n = npair * 2 + n2
                            nc.tensor.matmul(
                                ops2[n2][:],
                                lhsT=cth_tiles[(t, h)][:,
                                                       mm * 128:(mm + 1) * 128],
                                rhs=wo_sb[:, h * D + n * 512:
                                          h * D + (n + 1) * 512],
                                start=(h == 0),
                                stop=(h == GH - 1),
                            )
                    if mm == 3 and npair == 1:
                        for h in range(GH):
                            cth_tiles.pop((t, h))
                    ost = ostp.tile([128, 1024], BF16, name="ost", tag="ost")
                    if final:
                        # contiguous [128,1024] psum pair: one full-tile
                        # eviction, engines alternating
                        if (mm + npair) % 2 == 0:
                            nc.scalar.copy(ost[:], op[:])
                        else:
                            nc.vector.tensor_copy(ost[:], op[:])
                        # split across both queues so the tail drains fast
                        nc.sync.dma_start(
                            out=outp_out[t * BLK + mm * 128:
                                         t * BLK + (mm + 1) * 128,
                                         npair * 1024:npair * 1024 + 512],
                            in_=ost[:, 0:512],
                        )
                        nc.gpsimd.dma_start(
                            out=outp_out[t * BLK + mm * 128:
                                         t * BLK + (mm + 1) * 128,
                                         npair * 1024 + 512:
                                         (npair + 1) * 1024],
                            in_=ost[:, 512:1024],
                        )
                    else:
                        nc.scalar.copy(ost[:, 0:512], ops2[0][:])
                        nc.vector.tensor_copy(ost[:, 512:1024], ops2[1][:])
                        eng = nc.sync if npair == 0 else nc.gpsimd
                        eng.dma_start(
                            out=outp_out[t * BLK + mm * 128:
                                         t * BLK + (mm + 1) * 128,
                                         npair * 1024:(npair + 1) * 1024],
                            in_=ost[:],
                        )

                # -------- software pipeline --------
                # scores pairs 2-at-a-time, ctx in runs of 16 matmuls so the
                # PE weight-buffer pipeline stays dense within each class.
                # Out-proj units (8 matmuls each) are spread 2-per-step so
                # the PE stays the bottleneck on every step: bulk per-block
                # out-proj left ScalarE (8 exps = 8.9us) as the limiter on
                # the 3-of-4 steps that had no out-proj work.
                pending = []       # outproj units ready to emit
                for i in range(len(steps) + 1):
                    t, h = steps[i] if i < len(steps) else (None, None)
                    fresh = False  # block enqueued this step: its last cth
                    for jj in range(4):  # CAST is only one slot old at jj=1
                        if i < len(steps):
                            emit_scores_pair(i, 2 * jj)
                            emit_scores_pair(i, 2 * jj + 1)
                        if jj == 0 and i >= 2:
                            emit_transposes(i - 2)
                            tb, hb = steps[i - 2]
                            if hb == 3:
                                pending += [(tb, mm, npair)
                                            for mm in range(4)
                                            for npair in range(2)]
                                fresh = True
                        if jj == 0 and i == 1:
                            emit_mask_dmas(1, j2s=(3,))
                            # vp complete: dump raw bf16 (gating wait is
                            # already satisfied here, so the gpsimd queue
                            # is not blocked)
                            for half in range(2):
                                nc.gpsimd.dma_start(
                                    out=vpo_raw[:, half * KC * 516 // 2:
                                                (half + 1) * KC * 516 // 2],
                                    in_=vpo_sb[:, half * KC * 516 // 2:
                                               (half + 1) * KC * 516 // 2],
                                )
                        if jj == 1 and i < len(steps) and h == 2 and \
                                t + 1 < NB and t + 1 >= 2:
                            emit_mask_dmas(t + 1)
                        if jj == 3 or (jj == 1 and not fresh):
                            if pending:
                                emit_outproj_unit(*pending.pop(0))
                        if i >= 1:
                            emit_ctx_part(i - 1, 2 * jj)
                            emit_ctx_part(i - 1, 2 * jj + 1)
                # epilogue: transposes of the last step, remaining outproj
                # (block 3 plus any leftovers) through the freed psS pool
                emit_transposes(len(steps) - 1)
                pending += [(NB - 1, mm, npair)
                            for mm in range(4) for npair in range(2)]
                for unit in pending:
                    emit_outproj_unit(*unit, final=True)

    nc.compile()
    return nc


def get_nc():
    if "nc" not in _CACHE:
        _CACHE["nc"] = _build()
    return _CACHE["nc"]


def make_in_maps(inputs):
    q = np.asarray(inputs["q"], np.float32)
    k = np.asarray(inputs["k"], np.float32)
    v = np.asarray(inputs["v"], np.float32)
    mask = np.asarray(inputs["mask"])
    Wq = np.asarray(inputs["Wq"], np.float32)
    Wk = np.asarray(inputs["Wk"], np.float32)
    Wv = np.asarray(inputs["Wv"], np.float32)
    Wo = np.asarray(inputs["Wo"], np.float32)

    per_batch = []
    for b in range(B):
        maskTb = np.ascontiguousarray(
            (~mask[b].astype(bool)).T).astype(nbf16)  # [key, q]
        # tile (t, j2) = [128, 2048]: [j2*128+p, t*2048 + c2*512 + g]
        #   = maskTb[(4*j2+c2)*128 + p, t*512 + g]
        maskP = np.ascontiguousarray(
            maskTb.reshape(4, 4, 128, NB, 512)
            .transpose(0, 2, 3, 1, 4).reshape(S // 4, 4 * S))
        per_batch.append({
            "qT": np.ascontiguousarray(q[b].T).astype(nbf16),
            "kT": np.ascontiguousarray(k[b].T).astype(nbf16),
            "vT": np.ascontiguousarray(v[b].T).astype(nbf16),
            "maskP": maskP,
        })

    def packw(wT, ncols):
        # wT [rows, ncols] -> [128, (rows//128)*ncols] with chunk-major cols
        r = wT.shape[0] // 128
        return np.ascontiguousarray(
            wT.reshape(r, 128, ncols).transpose(1, 0, 2).reshape(128, r * ncols))

    per_group = []
    for g in range(4):
        sl = slice(g * GD, (g + 1) * GD)
        per_group.append({
            "wqP": packw(np.ascontiguousarray(Wq[sl, :].T).astype(nbf16), GD),
            "wkP": packw(np.ascontiguousarray(Wk[sl, :].T).astype(nbf16), GD),
            "wvP": packw(np.ascontiguousarray(Wv[sl, :].T).astype(nbf16), GD),
            "woP": packw(np.ascontiguousarray(Wo[:, sl].T).astype(nbf16), D),
        })
    in_maps = []
    for c in range(N_CORES):
        b, g = c // 4, c % 4
        m = {}
        m.update(per_batch[b])
        m.update(per_group[g])
        in_maps.append(m)
    return in_maps


def assemble(results):
    out = np.zeros((B, S, D), np.float32)
    kp = np.empty((B, S, D), np.float32)
    vp = np.empty((B, S, D), np.float32)
    for c, res in enumerate(results):
        b, g = c // 4, c % 4
        # kp_raw[p, m*2048 + s] = kp[s, g*512 + m*128 + p]
        kpr = np.asarray(res["kp_raw"]).astype(np.float32)
        kp[b][:, g * GD:(g + 1) * GD] = (
            kpr.reshape(128, 4, S).transpose(2, 1, 0).reshape(S, GD))
        # vpo_raw[p, sc*516 + h*129 + c] = vp[sc*128 + p, g*512 + h*128 + c]
        vpr = np.asarray(res["vpo_raw"]).astype(np.float32)
        vpr = vpr.reshape(128, KC, GH, 129)[:, :, :, :128]
        vp[b][:, g * GD:(g + 1) * GD] = (
            vpr.transpose(1, 0, 2, 3).reshape(S, GD))
        out[b] += res["outp_out"].astype(np.float32)
    return out, kp, vp


def run_cores(in_maps, trace=False, **kwargs):
    nc = get_nc()
    return bass_utils.run_bass_kernel_spmd(
        nc, in_maps, core_ids=list(range(N_CORES)), trace=trace, **kwargs
    )


def kernel(**inputs):
    in_maps = make_in_maps(inputs)
    res = run_cores(in_maps, trace=False)
    return assemble(res.results)


# revision 34
# speedup vs baseline: 1.0714x; 1.0093x over previous
"""Distributed Trainium2 kernel for nn_AttentionLayer (B=2, S=2048, D=2048, H=16).

Sharding: core c = (batch b, head-group g) with b = c // 4, g = c % 4.
Each core owns 4 heads (512 of the 2048 projection dims) of one batch element:
projections (bf16 matmuls, f32 accumulation), masked softmax attention for its
4 heads (no max-subtraction; masked entries become 0 via exp(s)*(1-mask)),
and its partial output projection (Wo row-shard). The 4 partial outputs per
batch are summed on the host (cross-core collectives hang on the axon PJRT
path in this container).

v3 changes (vs v2, driven by trace analysis: DMA front saturated at ~343GB/s
through t=50us with PE starving on activation arrival; 4us A->B boundary gap
from mask DMAs gated on stream-pool teardown; tail epilogue fixed):
- kp/vp are emitted as raw bf16 SBUF dumps (kp_raw straight from kp_sb,
  vpo_raw straight from vpo_sb); the host unpacks/casts. This removes all
  f32 staging copies and halves Phase-A outbound traffic.
- Phase A streams in exact consumption order: wq is interleaved with group-0
  activation tiles on sync/scalar; wk/wv/wo ride sync/scalar interleaved
  behind the NEXT group's activations (not gpsimd up-front), keeping the
  DMA-bound front window free of not-yet-needed bytes. gpsimd carries only
  outputs.
- Mask tiles are [128,2048] (4 per query block) in a dedicated pool created
  BEFORE the Phase-A pools, so mask DMAs are not gated on Phase-A teardown;
  blocks t=0,1 preload during Phase A's back half, t=2,3 load during Phase B.
- Phase B mask multiply is one 2048-wide DVE op per j-pair (2x 16-bit mode).
- Last v-group evictions alternate engines per-copy so the A->B boundary
  drains ~2x faster.
"""

import numpy as np
import ml_dtypes

import concourse.bass as bass  # noqa: F401
import concourse.mybir as mybir
import concourse.tile as tile
from concourse import bacc
from concourse import bass_utils
from concourse.masks import make_identity

BF16 = mybir.dt.bfloat16
F32 = mybir.dt.float32
nbf16 = ml_dtypes.bfloat16

B, S, D, H = 2, 2048, 2048, 16
GH = 4                # heads per core
DH = 128              # head dim
GD = GH * DH          # 512 local projection dims
KC = D // 128         # 16 contraction chunks
NB = 4                # query blocks
BLK = S // NB         # 512
NJ = KC // 2          # 8 skc-pairs per step
N_CORES = 8
SCALE = float(1.0 / np.sqrt(DH))

_CACHE = {}


def _build():
    nc = bacc.Bacc(
        "TRN2", target_bir_lowering=False, debug=False, num_devices=N_CORES
    )
    AF = mybir.ActivationFunctionType

    qT = nc.dram_tensor("qT", [D, S], BF16, kind="ExternalInput")
    kT = nc.dram_tensor("kT", [D, S], BF16, kind="ExternalInput")
    vT = nc.dram_tensor("vT", [D, S], BF16, kind="ExternalInput")
    # mask tile (t, j2) = [128, 2048]: col c2*512+g <-> key (4*j2+c2)*128+p,
    # query t*512+g (value 1.0 where attention allowed, 0.0 where masked)
    maskP = nc.dram_tensor("maskP", [S // 4, 4 * S], BF16, kind="ExternalInput")
    wqP = nc.dram_tensor("wqP", [128, KC * GD], BF16, kind="ExternalInput")
    wkP = nc.dram_tensor("wkP", [128, KC * GD], BF16, kind="ExternalInput")
    wvP = nc.dram_tensor("wvP", [128, KC * GD], BF16, kind="ExternalInput")
    woP = nc.dram_tensor("woP", [128, GH * D], BF16, kind="ExternalInput")
    kp_raw = nc.dram_tensor("kp_raw", [128, GH * S], BF16, kind="ExternalOutput")
    vpo_raw = nc.dram_tensor("vpo_raw", [128, KC * 516], BF16,
                             kind="ExternalOutput")
    outp_out = nc.dram_tensor("outp_out", [S, D], BF16, kind="ExternalOutput")

    with tile.TileContext(nc) as tc:
        with (
            tc.tile_pool(name="res", bufs=1) as res,
            tc.tile_pool(name="mpool", bufs=7) as mpool,
        ):
            # ---- resident SBUF tensors (live across both phases) ----
            wo_sb = res.tile([128, GH * D], BF16, name="wo_sb", tag="wo")
            qp_sb = res.tile([128, GH * S], BF16, name="qp_sb", tag="qp")
            kp_sb = res.tile([128, GH * S], BF16, name="kp_sb", tag="kp")
            # vp + per-head ones column: s-chunk sc at cols sc*516, head h at
            # +h*129 (128 vp dims then one 1.0 column for the softmax denom)
            vpo_sb = res.tile([128, KC * 516], BF16, name="vpo_sb", tag="vpo")
            ident = res.tile([128, 128], BF16, name="ident", tag="ident")

            mask_tiles = {}    # t -> list of 4 [128,2048] tiles

            def emit_mask_dmas(t, alternate=False, j2s=range(4)):
                mts = mask_tiles.setdefault(t, [])
                for j2 in j2s:
                    mt = mpool.tile([128, 2048], BF16, name="mt", tag="mt")
                    eng = nc.gpsimd if (alternate and j2 % 2 == 1) else nc.sync
                    eng.dma_start(
                        out=mt[:],
                        in_=maskP[j2 * 128:(j2 + 1) * 128,
                                  t * 2048:(t + 1) * 2048],
                    )
                    mts.append(mt)

            # ---------------- Phase A: projections ----------------
            with (
                tc.tile_pool(name="wpool", bufs=1) as wpool,
                tc.tile_pool(name="stream", bufs=41) as stream,
                tc.tile_pool(name="psA", bufs=8, space="PSUM") as psA,
            ):
                # wv is allocated lazily at group 2 sharing wq's TAG with
                # bufs=1: the ring reuses wq's buffer (wq's last consumer is
                # group 1's final matmul), freeing 16KB/partition for a
                # deeper stream pool
                W = {
                    "q": wpool.tile([128, KC * GD], BF16, name="wq_sb",
                                    tag="wqv"),
                    "k": wpool.tile([128, KC * GD], BF16, name="wk_sb",
                                    tag="wk"),
                }

                # Warm the PE clock gate (HAM) with dummy matmuls while the
                # first activation/weight DMAs are in flight: the PE would
                # otherwise idle ~10us and run its first ~3.4us of real
                # matmuls at half clock.
                wps = psA.tile([128, 512], F32, name="wps", tag="psa")
                for _ in range(56):
                    nc.tensor.matmul(
                        wps[:, 0:128], lhsT=ident[:], rhs=ident[:],
                        start=True, stop=True,
                    )

                make_identity(nc, ident[:])
                nc.vector.memset(vpo_sb[:], 1.0)

                # Projection groups, order q -> k -> v: q produces no output
                # traffic, so the DMA-saturated front window carries only
                # inputs; v's output burst (vpo_raw) lands at the end where
                # the inbound stream is light.
                groups = []
                for xk in ("q", "k", "v"):
                    for npair in range(2):
                        groups.append((xk, npair))
                xdram_of = {"v": vT, "k": kT, "q": qT}
                xss_of = {}

                # Weight/mask staging: emitted interleaved behind the act
                # tiles of the group listed here (queue-FIFO order == arrival
                # order; each consumer waits only for DMAs up to its own).
                # wk lands during g1 compute (needed at g2), wv during g2
                # (needed g4), wo during g3 (needed in phase B), masks t0/t1
                # during g4/g5 (needed at phase B steps 0/4).
                def wchunks(w_sb_, wP_, n):
                    step = (KC * GD) // n
                    return [(w_sb_[:, i * step:(i + 1) * step],
                             wP_[:, i * step:(i + 1) * step]) for i in range(n)]

                def extras_for(gi):
                    if gi == 2:
                        return wchunks(W["k"], wkP, 4)
                    if gi == 3:
                        W["v"] = wpool.tile([128, KC * GD], BF16,
                                            name="wv_sb", tag="wqv")
                        return wchunks(W["v"], wvP, 4)
                    if gi == 4:
                        return wchunks(wo_sb, woP, 4)
                    return []

                def issue_acts(gi):
                    xk, npair = groups[gi]
                    xd = xdram_of[xk]
                    xss = xss_of.setdefault(gi, [])
                    extras = extras_for(gi)
                    for kc in range(KC):
                        eng = nc.sync if kc % 2 == 0 else nc.gpsimd
                        xs = stream.tile([128, 1024], BF16, name="xs", tag="xs")
                        eng.dma_start(
                            out=xs[:],
                            in_=xd[kc * 128:(kc + 1) * 128,
                                   npair * 1024:(npair + 1) * 1024],
                        )
                        xss.append(xs)
                        # interleave a pending weight chunk every 4th tile
                        if kc % 4 == 3 and extras:
                            dst, src = extras.pop(0)
                            weng = nc.sync if (kc // 4) % 2 == 0 else nc.gpsimd
                            weng.dma_start(out=dst, in_=src)

                def emit_group(gi, xk, npair, xss):
                    # kc-outer over the whole group: 8 [128,512] psum chains
                    # (slot = m*2+h2 for q/k, sl = s-subchunk for v) accumulate
                    # together, consuming one act tile every ~1.7us.  Uniform
                    # ~220GB/s demand -- no per-chain 4.2MB burst, tiles
                    # release incrementally, the stream never falls behind.
                    pss = [psA.tile([128, 512], F32, name="ps", tag="psa")
                           for _ in range(8)]
                    for kc in range(KC):
                        for sl in range(8):
                            if xk == "v":
                                nc.tensor.matmul(
                                    pss[sl][:],
                                    lhsT=xss[kc][:, sl * 128:(sl + 1) * 128],
                                    rhs=W["v"][:, kc * GD:(kc + 1) * GD],
                                    start=(kc == 0),
                                    stop=(kc == KC - 1),
                                )
                            else:
                                m, h2 = sl // 2, sl % 2
                                wsb = W["k"] if xk == "k" else W["q"]
                                nc.tensor.matmul(
                                    pss[sl][:],
                                    lhsT=wsb[:, kc * GD + m * 128:
                                             kc * GD + (m + 1) * 128],
                                    rhs=xss[kc][:, h2 * 512:(h2 + 1) * 512],
                                    start=(kc == 0),
                                    stop=(kc == KC - 1),
                                )
                    # evictions in allocation order so the next group's psum
                    # reuse unblocks tile-by-tile
                    dst_bf = kp_sb if xk == "k" else qp_sb
                    for sl in range(8):
                        m, h2 = sl // 2, sl % 2
                        dst = dst_bf[:, m * S + npair * 1024 + h2 * 512:
                                     m * S + npair * 1024 + (h2 + 1) * 512]
                        if m % 2 == 0:
                            nc.scalar.copy(dst, pss[sl][:])
                        else:
                            nc.vector.tensor_copy(dst, pss[sl][:])

                def emit_v_chain(mqp, sp, xss):
                    # v groups run m-outer (their data is fully resident by
                    # then -- the stream is ~60us ahead) so the 32 vpo
                    # evictions stagger chain-by-chain instead of bunching
                    # after the group's last matmul, which would stall the
                    # phase boundary and phase B's first ctx matmuls.
                    ps2 = [psA.tile([128, 512], F32, name="ps", tag="psa")
                           for _ in range(2)]
                    for kc in range(KC):
                        for h2 in range(2):
                            sl = sp * 2 + h2
                            nc.tensor.matmul(
                                ps2[h2][:],
                                lhsT=xss[kc][:, sl * 128:(sl + 1) * 128],
                                rhs=W["v"][:, kc * GD:(kc + 1) * GD],
                                start=(kc == 0),
                                stop=(kc == KC - 1),
                            )
                    # whole-chain engine alternation: cross-engine writes to
                    # vpo_sb serialize (tile-granular WAW ordering)
                    for h2 in range(2):
                        sc = mqp * 8 + sp * 2 + h2
                        for h in range(GH):
                            dst = vpo_sb[:, sc * 516 + h * 129:
                                         sc * 516 + h * 129 + 128]
                            src = ps2[h2][:, h * 128:(h + 1) * 128]
                            if sp % 2 == 0:
                                nc.scalar.copy(dst, src)
                            else:
                                nc.vector.tensor_copy(dst, src)

                # group 0: wq chunks interleaved with its own act tiles in
                # exact consumption order (kc-outer chains below tolerate the
                # cold stream, consuming one tile every ~1.7us); kc-granular
                # wq chunks so the first matmul starts one chunk earlier
                xss0 = xss_of.setdefault(0, [])
                for kc in range(KC):
                    eng = nc.sync if kc % 2 == 0 else nc.gpsimd
                    if kc % 2 == 0:
                        # [128,1024] wq pair-chunk (2KB per partition row
                        # keeps DMA burst efficiency), queues alternating
                        weng = nc.sync if kc % 4 == 0 else nc.gpsimd
                        weng.dma_start(
                            out=W["q"][:, kc * GD:(kc + 2) * GD],
                            in_=wqP[:, kc * GD:(kc + 2) * GD],
                        )
                    xs = stream.tile([128, 1024], BF16, name="xs", tag="xs")
                    eng.dma_start(
                        out=xs[:],
                        in_=qT[kc * 128:(kc + 1) * 128, 0:1024],
                    )
                    xss0.append(xs)

                for gi, (xk, npair) in enumerate(groups):
                    xss = xss_of[gi]
                    if gi + 1 < len(groups):
                        issue_acts(gi + 1)
                    if gi == 4:
                        emit_mask_dmas(0, alternate=True)
                    if gi == 5:
                        # only 3 of block 1's mask tiles fit in mpool
                        # alongside block 0's; the 4th loads in phase B
                        emit_mask_dmas(1, alternate=True, j2s=range(3))
                    if xk == "v":
                        for sp in range(4):
                            emit_v_chain(npair, sp, xss)
                    else:
                        emit_group(gi, xk, npair, xss)
                    if xk == "k" and npair == 1:
                        # kp complete: dump raw bf16 (host unpacks);
                        # gpsimd queue is otherwise idle until phase B
                        for half in range(2):
                            nc.gpsimd.dma_start(
                                out=kp_raw[:, half * GH * S // 2:
                                           (half + 1) * GH * S // 2],
                                in_=kp_sb[:, half * GH * S // 2:
                                          (half + 1) * GH * S // 2],
                            )
                # (vpo_raw is dumped from phase B's pipeline: emitting it
                # here would park a long gating wait on the gpsimd queue)

            # ---------------- Phase B: attention + out-proj ----------------
            with (
                tc.tile_pool(name="apl", bufs=9) as apl,
                tc.tile_pool(name="cpl", bufs=10) as cpl,
                tc.tile_pool(name="stageB", bufs=12) as stageB,
                tc.tile_pool(name="ostp", bufs=6) as ostp,
                tc.tile_pool(name="psS", bufs=2, space="PSUM") as psS,
                tc.tile_pool(name="psCT", bufs=2, space="PSUM") as psCT,
                tc.tile_pool(name="psOT", bufs=2, space="PSUM") as psOT,
            ):
                steps = [(t, h) for t in range(NB) for h in range(GH)]
                at2_tiles = {}     # (i, j2) -> [128,2048] tile (skc 4j2..+3)
                cps_tiles = {}     # (i, jj//2) -> cps tile holding 2 chunks
                ctn_tiles = {}     # (i, jj) -> normalized ctx [128,128]
                cth_tiles = {}     # (t, h) -> [128,512] transposed ctx

                def emit_scores_pair(i, j):
                    t, h = steps[i]
                    sps = psS.tile([128, 1024], F32, name="sps", tag="sps")
                    for c in range(2):
                        skc = 2 * j + c
                        nc.tensor.matmul(
                            sps[:, c * 512:(c + 1) * 512],
                            lhsT=kp_sb[:, h * S + skc * 128:
                                       h * S + (skc + 1) * 128],
                            rhs=qp_sb[:, h * S + t * 512: h * S + (t + 1) * 512],
                            start=True,
                            stop=True,
                        )
                    j2 = j // 2
                    if j % 2 == 0:
                        at2_tiles[(i, j2)] = apl.tile(
                            [128, 2048], BF16, name="at2", tag="at2")
                    at2 = at2_tiles[(i, j2)]
                    half = at2[:, (j % 2) * 1024:(j % 2) * 1024 + 1024]
                    nc.scalar.activation(half, sps[:], AF.Exp, scale=SCALE)
                    if j % 2 == 1:
                        # one 2048-wide mask multiply per j-pair (DVE 2x mode)
                        nc.vector.tensor_mul(
                            at2[:], at2[:], mask_tiles[t][j2][:])

                def emit_ctx_part(i, j):
                    # ctx matmuls for step i, slice j: mm-chunk jj = j//2,
                    # skc range (j%2)*8 .. +8; finalize (recip+norm) at odd j.
                    t, h = steps[i]
                    jj = j // 2
                    if j % 2 == 0 and jj % 2 == 0:
                        cps_tiles[(i, jj // 2)] = psCT.tile(
                            [128, 512], F32, name="cps", tag="cps")
                    cps = cps_tiles[(i, jj // 2)]
                    off = (jj % 2) * 129
                    for skc in range((j % 2) * 8, (j % 2) * 8 + 8):
                        at2 = at2_tiles[(i, skc // 4)]
                        nc.tensor.matmul(
                            cps[:, off:off + 129],
                            lhsT=at2[:, (skc % 4) * 512 + jj * 128:
                                     (skc % 4) * 512 + (jj + 1) * 128],
                            rhs=vpo_sb[:, skc * 516 + h * 129:
                                       skc * 516 + (h + 1) * 129],
                            start=(skc == 0),
                            stop=(skc == KC - 1),
                        )
                    if j % 2 == 1:
                        rec = stageB.tile([128, 1], F32, name="rec", tag="rec")
                        nc.vector.reciprocal(rec[:], cps[:, off + 128:off + 129])
                        ctn = stageB.tile([128, 128], BF16, name="ctn",
                                          tag="ctn")
                        nc.vector.tensor_scalar_mul(
                            ctn[:], cps[:, off:off + 128], rec[:])
                        ctn_tiles[(i, jj)] = ctn
                    if j == 7:
                        # release the at2 tiles of step i
                        for jd in range(4):
                            at2_tiles.pop((i, jd), None)

                def emit_transposes(i):
                    t, h = steps[i]
                    tps = psOT.tile([128, 512], BF16, name="tps", tag="psot")
                    for mm in range(4):
                        nc.tensor.transpose(
                            tps[:, mm * 128:(mm + 1) * 128],
                            ctn_tiles.pop((i, mm))[:], ident[:])
                    cth = cpl.tile([128, 512], BF16, name="cth", tag="cth")
                    nc.vector.tensor_copy(cth[:], tps[:])
                    cth_tiles[(t, h)] = cth

                def emit_outproj_unit(t, mm, npair, final=False):
                    # one (mm, npair) unit: 8 matmuls accumulating over the
                    # 4 heads into a [128,1024] output row-block slice
                    if final:
                        # scores are done: run through the freed psS pool
                        # ([128,1024] pairs, 2-deep) so units pipeline
                        op = psS.tile([128, 1024], F32, name="opw", tag="sps")
                        ops2 = [op[:, 0:512], op[:, 512:1024]]
                    else:
                        ops2 = [psOT.tile([128, 512], F32,
                                          name=f"ops{j2}", tag="psot")
                                for j2 in range(2)]
                    for h in range(GH):
                        for n2 in range(2):
                            n = npair * 2 + n2
                            nc.tensor.matmul(
                                ops2[n2][:],
                                lhsT=cth_tiles[(t, h)][:,
                                                       mm * 128:(mm + 1) * 128],
                                rhs=wo_sb[:, h * D + n * 512:
                                          h * D + (n + 1) * 512],
                                start=(h == 0),
                                stop=(h == GH - 1),
                            )
                    if mm == 3 and npair == 1:
                        for h in range(GH):
                            cth_tiles.pop((t, h))
                    ost = ostp.tile([128, 1024], BF16, name="ost", tag="ost")
                    if final:
                        # contiguous [128,1024] psum pair: one full-tile
                        # eviction, engines alternating
                        if (mm + npair) % 2 == 0:
                            nc.scalar.copy(ost[:], op[:])
                        else:
                            nc.vector.tensor_copy(ost[:], op[:])
                        # whole-tile queue alternation: each queue drains
                        # its half of the tail independently (a half-split
                        # per tile made the tail wait on the slower
                        # software-dynamic gpsimd queue for every tile)
                        eng = nc.sync if (mm + npair) % 2 == 0 else nc.gpsimd
                        eng.dma_start(
                            out=outp_out[t * BLK + mm * 128:
                                         t * BLK + (mm + 1) * 128,
                                         npair * 1024:(npair + 1) * 1024],
                            in_=ost[:],
                        )
                    else:
                        nc.scalar.copy(ost[:, 0:512], ops2[0][:])
                        nc.vector.tensor_copy(ost[:, 512:1024], ops2[1][:])
                        eng = nc.sync if npair == 0 else nc.gpsimd
                        eng.dma_start(
                            out=outp_out[t * BLK + mm * 128:
                                         t * BLK + (mm + 1) * 128,
                                         npair * 1024:(npair + 1) * 1024],
                            in_=ost[:],
                        )

                # -------- software pipeline --------
                # scores pairs 2-at-a-time, ctx in runs of 16 matmuls so the
                # PE weight-buffer pipeline stays dense within each class.
                # Out-proj units (8 matmuls each) are spread 2-per-step so
                # the PE stays the bottleneck on every step: bulk per-block
                # out-proj left ScalarE (8 exps = 8.9us) as the limiter on
                # the 3-of-4 steps that had no out-proj work.
                pending = []       # outproj units ready to emit
                for i in range(len(steps) + 1):
                    t, h = steps[i] if i < len(steps) else (None, None)
                    fresh = False  # block enqueued this step: its last cth
                    for jj in range(4):  # CAST is only one slot old at jj=1
                        if i < len(steps):
                            emit_scores_pair(i, 2 * jj)
                            emit_scores_pair(i, 2 * jj + 1)
                        if jj == 0 and i >= 2:
                            emit_transposes(i - 2)
                            tb, hb = steps[i - 2]
                            if hb == 3:
                                pending += [(tb, mm, npair)
                                            for mm in range(4)
                                            for npair in range(2)]
                                fresh = True
                        if jj == 0 and i == 1:
                            emit_mask_dmas(1, j2s=(3,))
                            # vp complete: dump raw bf16 (gating wait is
                            # already satisfied here, so the gpsimd queue
                            # is not blocked)
                            for half in range(2):
                                nc.gpsimd.dma_start(
                                    out=vpo_raw[:, half * KC * 516 // 2:
                                                (half + 1) * KC * 516 // 2],
                                    in_=vpo_sb[:, half * KC * 516 // 2:
                                               (half + 1) * KC * 516 // 2],
                                )
                        if jj == 1 and i < len(steps) and h == 2 and \
                                t + 1 < NB and t + 1 >= 2:
                            emit_mask_dmas(t + 1)
                        if jj == 3 or (jj in (1, 2) and not fresh):
                            # up to 3 units/step keeps the epilogue down to
                            # one block (the tail has nothing to overlap)
                            if pending:
                                emit_outproj_unit(*pending.pop(0))
                        if i >= 1:
                            emit_ctx_part(i - 1, 2 * jj)
                            emit_ctx_part(i - 1, 2 * jj + 1)
                # epilogue: transposes of the last step, remaining outproj
                # (block 3 plus any leftovers) through the freed psS pool
                emit_transposes(len(steps) - 1)
                pending += [(NB - 1, mm, npair)
                            for mm in range(4) for npair in range(2)]
                for unit in pending:
                    emit_outproj_unit(*unit, final=True)

    nc.compile()
    return nc


def get_nc():
    if "nc" not in _CACHE:
        _CACHE["nc"] = _build()
    return _CACHE["nc"]


def make_in_maps(inputs):
    q = np.asarray(inputs["q"], np.float32)
    k = np.asarray(inputs["k"], np.float32)
    v = np.asarray(inputs["v"], np.float32)
    mask = np.asarray(inputs["mask"])
    Wq = np.asarray(inputs["Wq"], np.float32)
    Wk = np.asarray(inputs["Wk"], np.float32)
    Wv = np.asarray(inputs["Wv"], np.float32)
    Wo = np.asarray(inputs["Wo"], np.float32)

    per_batch = []
    for b in range(B):
        maskTb = np.ascontiguousarray(
            (~mask[b].astype(bool)).T).astype(nbf16)  # [key, q]
        # tile (t, j2) = [128, 2048]: [j2*128+p, t*2048 + c2*512 + g]
        #   = maskTb[(4*j2+c2)*128 + p, t*512 + g]
        maskP = np.ascontiguousarray(
            maskTb.reshape(4, 4, 128, NB, 512)
            .transpose(0, 2, 3, 1, 4).reshape(S // 4, 4 * S))
        per_batch.append({
            "qT": np.ascontiguousarray(q[b].T).astype(nbf16),
            "kT": np.ascontiguousarray(k[b].T).astype(nbf16),
            "vT": np.ascontiguousarray(v[b].T).astype(nbf16),
            "maskP": maskP,
        })

    def packw(wT, ncols):
        # wT [rows, ncols] -> [128, (rows//128)*ncols] with chunk-major cols
        r = wT.shape[0] // 128
        return np.ascontiguousarray(
            wT.reshape(r, 128, ncols).transpose(1, 0, 2).reshape(128, r * ncols))

    per_group = []
    for g in range(4):
        sl = slice(g * GD, (g + 1) * GD)
        per_group.append({
            "wqP": packw(np.ascontiguousarray(Wq[sl, :].T).astype(nbf16), GD),
            "wkP": packw(np.ascontiguousarray(Wk[sl, :].T).astype(nbf16), GD),
            "wvP": packw(np.ascontiguousarray(Wv[sl, :].T).astype(nbf16), GD),
            "woP": packw(np.ascontiguousarray(Wo[:, sl].T).astype(nbf16), D),
        })
    in_maps = []
    for c in range(N_CORES):
        b, g = c // 4, c % 4
        m = {}
        m.update(per_batch[b])
        m.update(per_group[g])
        in_maps.append(m)
    return in_maps


def assemble(results):
    out = np.zeros((B, S, D), np.float32)
    kp = np.empty((B, S, D), np.float32)
    vp = np.empty((B, S, D), np.float32)
    for c, res in enumerate(results):
        b, g = c // 4, c % 4
        # kp_raw[p, m*2048 + s] = kp[s, g*512 + m*128 + p]
        kpr = np.asarray(res["kp_raw"]).astype(np.float32)
        kp[b][:, g * GD:(g + 1) * GD] = (
            kpr.reshape(128, 4, S).transpose(2, 1, 0).reshape(S, GD))
        # vpo_raw[p, sc*516 + h*129 + c] = vp[sc*128 + p, g*512 + h*128 + c]
        vpr = np.asarray(res["vpo_raw"]).astype(np.float32)
        vpr = vpr.reshape(128, KC, GH, 129)[:, :, :, :128]
        vp[b][:, g * GD:(g + 1) * GD] = (
            vpr.transpose(1, 0, 2, 3).reshape(S, GD))
        out[b] += res["outp_out"].astype(np.float32)
    return out, kp, vp


def run_cores(in_maps, trace=False, **kwargs):
    nc = get_nc()
    return bass_utils.run_bass_kernel_spmd(
        nc, in_maps, core_ids=list(range(N_CORES)), trace=trace, **kwargs
    )


def kernel(**inputs):
    in_maps = make_in_maps(inputs)
    res = run_cores(in_maps, trace=False)
    return assemble(res.results)
